# revision 1
# baseline (speedup 1.0000x reference)
"""BinaryXnorExceptOutliersLinear on 8 Trainium2 NeuronCores.

Reference math:
    mask, bscale from global kth-value quantiles of w
    w_q  = per-row asymmetric 8-bit fake quant of w
    w_sim = mask ? w_q : sign(w_q)*bscale
    out  = x @ w_sim.T + bias

Strategy: the weight transform is data-independent of x, so it is done on
the host (numpy, f32, op-for-op like the reference). The device kernel is
a pure streaming GEMM over an fp8(e3m4) encoding of w_sim: per out-row o,
codes = w_sim[o,:]/s_o with s_o = bscale/nb_o and nb_o the largest
e3m4-exact value such that max|codes| <= 15.5. Non-outliers (+-bscale,
95% of weights) encode EXACTLY as +-nb_o; only outliers carry e3m4
rounding (~3%), giving ~7e-3 output rel err. Each core streams its
pre-transposed [8192 in, 1024 out] fp8 shard (8 MiB) as the PE moving
operand against stationary f16 x chunks, accumulating in two 512-wide
PSUM banks; the host applies s_o per column, adds bias, and concatenates
the 8 shards.
"""
import sys

sys.path.insert(0, "/opt/trn_rl_repo")

import numpy as np
import ml_dtypes
from contextlib import ExitStack

import bass_rust
import concourse.bass as bass
import concourse.mybir as mybir
import concourse.tile as tile
from concourse.bass_utils import run_bass_kernel_spmd

# ---------------------------------------------------------------------------
OUT_F = 8192
IN_F = 8192
BATCH = 32
N_CORES = 8
ROWS_PER_CORE = OUT_F // N_CORES      # 1024
P = 128
CH = IN_F // P                         # 64 contraction chunks
OUTLIER_FRACTION = 0.05
F8MAX = 15.5                           # e3m4 max finite

f32 = mybir.dt.float32
f16 = mybir.dt.float16
f8 = mybir.dt.float8e3

# ---------------------------------------------------------------------------
# walrus compatibility


def _prepare_for_walrus(nc):
    mybir.codegen_inst_isa_subclasses(nc)
    ctr = 0
    for bb in nc.main_func.blocks:
        new = []
        for inst in bb.instructions:
            si = inst.sync_info
            if si is not None and len(si.on_wait) > 1:
                waits = list(si.on_wait)
                for w in waits[:-1]:
                    nop = bass_rust.InstNoOp(
                        name=f"I-wsplit-{ctr}", engine=inst.engine
                    )
                    ctr += 1
                    nop.sync_info = mybir.SyncInfo(on_wait=[w], on_update=[])
                    try:
                        nc.register_instruction(nop, overwrite=True)
                    except Exception:
                        pass
                    new.append(nop)
                si.on_wait = [waits[-1]]
            new.append(inst)
        bb.instructions = new
    return nc


# ---------------------------------------------------------------------------
# device program: psum[32, 1024] = x16[32, 8192] @ codes[8192, 1024]

# staged weight-stream schedule (units of 128-row contraction chunks):
# small chunks first for an early PE start, 1 MiB chunks after for DMA
# efficiency; big pool fully resident so DMA never stalls on reuse
SCHED = [1, 1, 1, 1, 2, 2] + [4] * 14
assert sum(SCHED) == CH


def _build_nc():
    nc = bass.Bass()
    # host layout: wP[p, c, o] = codesT[c*128 + p, o] -> contiguous lines
    wP = nc.dram_tensor("wP", [P, CH * ROWS_PER_CORE], f8,
                        kind="ExternalInput")
    xS = nc.dram_tensor("xS", [P, CH * BATCH], f16, kind="ExternalInput")
    y = nc.dram_tensor("y", [BATCH, ROWS_PER_CORE], f32, kind="ExternalOutput")

    A = mybir.AluOpType

    with tile.TileContext(nc) as tc, ExitStack() as ctx:
        const_pool = ctx.enter_context(tc.tile_pool(name="const", bufs=1))
        pool_s = ctx.enter_context(tc.tile_pool(name="ws", bufs=4))
        pool_m2 = ctx.enter_context(tc.tile_pool(name="wm2", bufs=2))
        pool_m4 = ctx.enter_context(tc.tile_pool(name="wm4", bufs=14))
        psum = ctx.enter_context(tc.tile_pool(name="psum", bufs=1, space="PSUM"))

        # x, host-laid-out as [p, c, b], in three pieces: a tiny 32 KiB
        # leader so c0's matmuls gate on minimal bytes, the rest slotted
        # into the queues behind the early weight chunks
        xt = const_pool.tile([P, CH, BATCH], f16)
        nc.sync.dma_start(xt[:, 0:4, :], xS[:, 0:4 * BATCH])

        ps0 = psum.tile([BATCH, 512], f32, tag="ps0")
        ps1 = psum.tile([BATCH, 512], f32, tag="ps1")

        c = 0
        for k, w in enumerate(SCHED):
            pool = {1: pool_s, 2: pool_m2, 4: pool_m4}[w]
            wt = pool.tile([P, w, ROWS_PER_CORE], f8)
            eng = nc.sync if k % 2 == 0 else nc.scalar
            eng.dma_start(
                wt[:],
                wP[:, c * ROWS_PER_CORE:(c + w) * ROWS_PER_CORE],
            )
            if k == 3:
                nc.scalar.dma_start(xt[:, 4:32, :],
                                    xS[:, 4 * BATCH:32 * BATCH])
            elif k == 7:
                nc.scalar.dma_start(xt[:, 32:CH, :],
                                    xS[:, 32 * BATCH:CH * BATCH])
            last = (k == len(SCHED) - 1)
            if last:
                # finish ps0 first so its copy overlaps ps1's last matmuls
                for j in range(w):
                    cc = c + j
                    nc.tensor.matmul(ps0[:], xt[:, cc, :], wt[:, j, 0:512],
                                     start=(cc == 0), stop=(cc == CH - 1))
                for j in range(w):
                    cc = c + j
                    nc.tensor.matmul(ps1[:], xt[:, cc, :], wt[:, j, 512:1024],
                                     start=(cc == 0), stop=(cc == CH - 1))
                c += w
            else:
                for j in range(w):
                    st, sp = (c == 0), False
                    nc.tensor.matmul(ps0[:], xt[:, c, :], wt[:, j, 0:512],
                                     start=st, stop=sp)
                    nc.tensor.matmul(ps1[:], xt[:, c, :], wt[:, j, 512:1024],
                                     start=st, stop=sp)
                    c += 1

        opool = ctx.enter_context(tc.tile_pool(name="o", bufs=1))
        ot = opool.tile([BATCH, ROWS_PER_CORE], f32)
        nc.scalar.copy(ot[:, 0:512], ps0[:])
        nc.sync.dma_start(y[:, 0:512], ot[:, 0:512])
        nc.vector.tensor_scalar(ot[:, 512:768], ps1[:, 0:256], 0.0, None,
                                A.add)
        nc.sync.dma_start(y[:, 512:768], ot[:, 512:768])
        nc.scalar.copy(ot[:, 768:1024], ps1[:, 256:512])
        nc.scalar.dma_start(y[:, 768:1024], ot[:, 768:1024])

    _prepare_for_walrus(nc)
    return nc


_NC_CACHE = None


def _get_nc():
    global _NC_CACHE
    if _NC_CACHE is None:
        _NC_CACHE = _build_nc()
    return _NC_CACHE


# ---------------------------------------------------------------------------
# host precompute: reference weight transform + e3m4 encoding


def _host_wsim(weight):
    w = np.ascontiguousarray(weight, dtype=np.float32)
    n = w.size
    k_lo = int(n * OUTLIER_FRACTION / 2)
    k_hi = int(n * (1.0 - OUTLIER_FRACTION / 2))
    part = np.partition(w.reshape(-1), [k_lo - 1, k_hi - 1])
    lo = np.float32(part[k_lo - 1])
    hi = np.float32(part[k_hi - 1])
    mask = (w < lo) | (w > hi)
    keep = ~mask
    bscale = np.float32(
        np.sum(np.abs(w) * keep, dtype=np.float32)
        / np.sum(keep, dtype=np.float32)
    )
    # per-row asymmetric 8-bit fake quant, f32 op-for-op like the reference
    w_min = w.min(1, keepdims=True).astype(np.float32)
    w_max = w.max(1, keepdims=True).astype(np.float32)
    rng = (w_max - w_min).astype(np.float32)
    zp = np.round(w_min - np.float32(128.0) * rng / np.float32(255.0)).astype(
        np.float32)
    q = (w - zp).astype(np.float32)
    q = (q * np.float32(255.0)).astype(np.float32)
    q = (q / rng).astype(np.float32)
    q = np.clip(np.round(q), np.float32(0.0), np.float32(255.0)).astype(
        np.float32)
    w_q = (q * (rng / np.float32(255.0)) + zp).astype(np.float32)
    w_sim = np.where(mask, w_q, np.sign(w_q) * bscale).astype(np.float32)
    return w_sim, bscale


def _snap_down_e3m4(v):
    """Largest e3m4-exact value <= v (v positive)."""
    c = v.astype(ml_dtypes.float8_e3m4)
    cf = c.astype(np.float32)
    bits = c.view(np.uint8)
    bits = np.where(cf > v, bits - 1, bits)
    return bits.view(ml_dtypes.float8_e3m4).astype(np.float32)


def _encode_e3m4(w_sim, bscale):
    M = np.abs(w_sim).max(1)
    nb_t = (np.float32(F8MAX) * bscale / M * np.float32(0.999)).astype(
        np.float32)
    nb = _snap_down_e3m4(nb_t)
    s = (bscale / nb).astype(np.float32)
    codes_f = np.clip(w_sim / s[:, None], -F8MAX, F8MAX)
    codes = codes_f.astype(ml_dtypes.float8_e3m4)
    return codes, s


def _run(inputs, trace=False):
    x, weight, bias = inputs["x"], inputs["weight"], inputs["bias"]
    w_sim, bscale = _host_wsim(weight)
    codes, s = _encode_e3m4(w_sim, bscale)

    x2 = np.ascontiguousarray(x, dtype=np.float32).reshape(BATCH, IN_F)
    # [p, c, b] layout: in-feature i = c*128 + p
    xS = np.ascontiguousarray(
        x2.T.reshape(CH, P, BATCH).transpose(1, 0, 2).reshape(P, CH * BATCH)
    ).astype(np.float16)

    nc = _get_nc()
    in_maps = []
    for c in range(N_CORES):
        sl = slice(c * ROWS_PER_CORE, (c + 1) * ROWS_PER_CORE)
        # [in, out] -> [p, c, o] with in = c*128 + p
        cT = codes[sl].T.reshape(CH, P, ROWS_PER_CORE)
        wPc = np.ascontiguousarray(cT.transpose(1, 0, 2)).reshape(
            P, CH * ROWS_PER_CORE)
        in_maps.append({
            "wP": wPc,
            "xS": xS,
        })
    res = run_bass_kernel_spmd(
        nc, in_maps, core_ids=list(range(N_CORES)), trace=trace
    )
    ys = np.concatenate([r["y"] for r in res.results], axis=1)  # [32, 8192]
    out = (ys * s[None, :] + np.asarray(bias, np.float32)[None, :]).reshape(
        BATCH, 1, OUT_F).astype(np.float32)
    return out, res


def kernel(**inputs):
    out, _ = _run(inputs, trace=False)
    return out



# revision 2
# speedup vs baseline: 1.1892x; 1.1892x over previous
"""BinaryXnorExceptOutliersLinear on 8 Trainium2 NeuronCores.

Reference math:
    mask, bscale from global kth-value quantiles of w
    w_q  = per-row asymmetric 8-bit fake quant of w
    w_sim = mask ? w_q : sign(w_q)*bscale
    out  = x @ w_sim.T + bias

Strategy: the weight transform is data-independent of x, so it is done on
the host (numpy, f32, op-for-op like the reference). The device kernel is
a DoubleRow fp8(e4m3) GEMM: per out-row o, codes = w_sim[o,:]/s_o with
s_o = bscale/nb_o and nb_o the largest e4m3-exact value such that
max|codes| <= 240. Non-outliers (+-bscale, 95% of weights) encode EXACTLY
as +-nb_o; only outliers carry e4m3 rounding (~1.3e-2 output rel err).
x is split hi+lo into two fp8 parts packed side by side in the stationary
operand (64 of 128 stationary columns -> DoubleRow max), recovering ~fp16
x precision at zero PE cost. Each core holds its full 8 MiB weight shard
resident in SBUF (64 KiB/partition), streamed via large dependency-free
DMAs on both HW DGE queues; matmuls chase the stream per 256-row k-pair
chunk. Output columns are processed in 2 groups of 512 so group 0's
PSUM drain (DVE copy + DMA out) overlaps group 1's matmuls.
"""
import sys

sys.path.insert(0, "/opt/trn_rl_repo")

import numpy as np
import ml_dtypes
from contextlib import ExitStack

import bass_rust
import concourse.bass as bass
import concourse.mybir as mybir
import concourse.tile as tile
from concourse.bass_utils import run_bass_kernel_spmd

# ---------------------------------------------------------------------------
OUT_F = 8192
IN_F = 8192
BATCH = 32
N_CORES = 8
ROWS_PER_CORE = OUT_F // N_CORES      # 1024
P = 128
CP = IN_F // (2 * P)                   # 32 k-pair chunks of 256
G = 2                                  # output column groups per core
GN = ROWS_PER_CORE // G                # 512 cols per group
OUTLIER_FRACTION = 0.05
F8MAX = 240.0                          # trn float8e4 (IEEE e4m3) max normal

f32 = mybir.dt.float32
f8 = mybir.dt.float8e4
F8NP = ml_dtypes.float8_e4m3

# ---------------------------------------------------------------------------
# walrus compatibility


def _prepare_for_walrus(nc):
    mybir.codegen_inst_isa_subclasses(nc)
    ctr = 0
    for bb in nc.main_func.blocks:
        new = []
        for inst in bb.instructions:
            si = inst.sync_info
            if si is not None and len(si.on_wait) > 1:
                waits = list(si.on_wait)
                for w in waits[:-1]:
                    nop = bass_rust.InstNoOp(
                        name=f"I-wsplit-{ctr}", engine=inst.engine
                    )
                    ctr += 1
                    nop.sync_info = mybir.SyncInfo(on_wait=[w], on_update=[])
                    try:
                        nc.register_instruction(nop, overwrite=True)
                    except Exception:
                        pass
                    new.append(nop)
                si.on_wait = [waits[-1]]
            new.append(inst)
        bb.instructions = new
    return nc


# ---------------------------------------------------------------------------
# device program
#
# psum_g[64, 512] = sum_c  xt[:, c].T @ wt[:, g*32+c]   (DoubleRow, K=256/chunk)
#   stationary xt chunk [128, 2, 64]: cols 0:32 x_hi, 32:64 x_lo
#   moving    wt chunk [128, 2, 512]
# y[64, 1024]: rows 0:32 hi part, 32:64 lo part; host adds halves, applies
# per-col scale s and bias.

# weight DMA pieces in units of 128 KiB k-pair chunks; small head for an
# early PE start, small tail so the last matmuls/drain start ASAP
SCHEDW = [1, 1, 2, 2, 4, 4, 6, 8, 8, 8, 8, 6, 4, 2]
assert sum(SCHEDW) == G * CP

XSPLIT = [8, 24]                       # x DMA pieces (k-pair chunks)


def _build_nc():
    nc = bass.Bass()
    # host layouts (per-partition contiguous):
    #   wP[p, (g*32+c)*1024 + i*512 + n] = code[g*512+n, c*256+i*128+p]
    #   xS[p, c*128 + i*64 + m]          = xhl[m, c*256+i*128+p]
    wP = nc.dram_tensor("wP", [P, G * CP * 2 * GN], f8, kind="ExternalInput")
    xS = nc.dram_tensor("xS", [P, CP * 2 * 64], f8, kind="ExternalInput")
    y = nc.dram_tensor("y", [64, ROWS_PER_CORE], f32, kind="ExternalOutput")

    PM = mybir.MatmulPerfMode.DoubleRow
    A = mybir.AluOpType

    with tile.TileContext(nc) as tc, ExitStack() as ctx:
        xpool = ctx.enter_context(tc.tile_pool(name="x", bufs=len(XSPLIT)))
        wpool = ctx.enter_context(tc.tile_pool(name="w", bufs=len(SCHEDW)))
        opool = ctx.enter_context(tc.tile_pool(name="o", bufs=G))
        psum = ctx.enter_context(tc.tile_pool(name="ps", bufs=G, space="PSUM"))

        # x pieces: first small piece unblocks chunk 0 quickly
        xts = []
        xoff = 0
        for k, nx in enumerate(XSPLIT):
            xt = xpool.tile([P, nx, 2, 64], f8)
            eng = nc.sync if k == 0 else nc.scalar
            eng.dma_start(xt[:], xS[:, xoff * 128:(xoff + nx) * 128])
            xts.append((xoff, xoff + nx, xt))
            xoff += nx

        # weight pieces: dependency-free, alternate HW DGE queues
        wts = []           # (gc_start, gc_end, tile)
        c = 0
        for k, wn in enumerate(SCHEDW):
            wt = wpool.tile([P, wn, 2, GN], f8)
            eng = nc.sync if k % 2 == 0 else nc.scalar
            eng.dma_start(wt[:], wP[:, c * 2 * GN:(c + wn) * 2 * GN])
            wts.append((c, c + wn, wt))
            c += wn

        def xchunk(c):
            for a, b, xt in xts:
                if a <= c < b:
                    return xt[:, c - a]
            raise AssertionError

        def wchunk(gc):
            for a, b, wt in wts:
                if a <= gc < b:
                    return wt[:, gc - a]
            raise AssertionError

        for g in range(G):
            ps = psum.tile([64, GN], f32, tag=f"ps{g}")
            for c in range(CP):
                nc.tensor.matmul(ps[:], xchunk(c), wchunk(g * CP + c),
                                 start=(c == 0), stop=(c == CP - 1),
                                 perf_mode=PM)
            ot = opool.tile([64, GN], f32, tag=f"ot{g}")
            nc.vector.tensor_scalar(ot[:], ps[:], 0.0, None, A.add)
            eng = nc.sync if g % 2 == 0 else nc.scalar
            eng.dma_start(y[:, g * GN:(g + 1) * GN], ot[:])

    _prepare_for_walrus(nc)
    return nc


_NC_CACHE = None


def _get_nc():
    global _NC_CACHE
    if _NC_CACHE is None:
        _NC_CACHE = _build_nc()
    return _NC_CACHE


# ---------------------------------------------------------------------------
# host precompute: reference weight transform + e4m3 encoding


def _host_wsim(weight):
    w = np.ascontiguousarray(weight, dtype=np.float32)
    n = w.size
    k_lo = int(n * OUTLIER_FRACTION / 2)
    k_hi = int(n * (1.0 - OUTLIER_FRACTION / 2))
    part = np.partition(w.reshape(-1), [k_lo - 1, k_hi - 1])
    lo = np.float32(part[k_lo - 1])
    hi = np.float32(part[k_hi - 1])
    mask = (w < lo) | (w > hi)
    keep = ~mask
    bscale = np.float32(
        np.sum(np.abs(w) * keep, dtype=np.float32)
        / np.sum(keep, dtype=np.float32)
    )
    # per-row asymmetric 8-bit fake quant, f32 op-for-op like the reference
    w_min = w.min(1, keepdims=True).astype(np.float32)
    w_max = w.max(1, keepdims=True).astype(np.float32)
    rng = (w_max - w_min).astype(np.float32)
    zp = np.round(w_min - np.float32(128.0) * rng / np.float32(255.0)).astype(
        np.float32)
    q = (w - zp).astype(np.float32)
    q = (q * np.float32(255.0)).astype(np.float32)
    q = (q / rng).astype(np.float32)
    q = np.clip(np.round(q), np.float32(0.0), np.float32(255.0)).astype(
        np.float32)
    w_q = (q * (rng / np.float32(255.0)) + zp).astype(np.float32)
    w_sim = np.where(mask, w_q, np.sign(w_q) * bscale).astype(np.float32)
    return w_sim, bscale


def _snap_down_f8(v):
    """Largest e4m3-exact value <= v (v positive normal)."""
    c = v.astype(F8NP)
    cf = c.astype(np.float32)
    bits = c.view(np.uint8)
    bits = np.where(cf > v, bits - 1, bits)
    return bits.view(F8NP).astype(np.float32)


def _encode_f8(w_sim, bscale):
    M = np.abs(w_sim).max(1)
    nb_t = (np.float32(F8MAX) * bscale / M * np.float32(0.999)).astype(
        np.float32)
    nb = _snap_down_f8(nb_t)
    s = (bscale / nb).astype(np.float32)
    codes = np.clip(w_sim / s[:, None], -F8MAX, F8MAX).astype(F8NP)
    return codes, s


def _run(inputs, trace=False):
    x, weight, bias = inputs["x"], inputs["weight"], inputs["bias"]
    w_sim, bscale = _host_wsim(weight)
    codes, s = _encode_f8(w_sim, bscale)

    x2 = np.ascontiguousarray(x, dtype=np.float32).reshape(BATCH, IN_F)
    x_hi = x2.astype(F8NP).astype(np.float32)
    x_lo = (x2 - x_hi).astype(F8NP)
    # xS[p, c*128 + i*64 + m]; m = h*32 + b; k = c*256 + i*128 + p
    st = np.stack([x_hi.astype(F8NP), x_lo], axis=0)   # [h, b, k]
    st = st.reshape(2, BATCH, CP, 2, P)                # [h, b, c, i, p]
    xSv = np.ascontiguousarray(st.transpose(4, 2, 3, 0, 1)).reshape(
        P, CP * 2 * 64)

    nc = _get_nc()
    in_maps = []
    for cid in range(N_CORES):
        sl = slice(cid * ROWS_PER_CORE, (cid + 1) * ROWS_PER_CORE)
        # wP[p, g, c, i, n] = codes_core[g*512+n, c*256+i*128+p]
        cc = codes[sl].reshape(G, GN, CP, 2, P)         # [g, n, c, i, p]
        wPc = np.ascontiguousarray(cc.transpose(4, 0, 2, 3, 1)).reshape(
            P, G * CP * 2 * GN)
        in_maps.append({"wP": wPc, "xS": xSv})
    res = run_bass_kernel_spmd(
        nc, in_maps, core_ids=list(range(N_CORES)), trace=trace
    )
    ys = np.concatenate([r["y"][0:32] + r["y"][32:64] for r in res.results],
                        axis=1)                          # [32, 8192]
    out = (ys * s[None, :] + np.asarray(bias, np.float32)[None, :]).reshape(
        BATCH, 1, OUT_F).astype(np.float32)
    return out, res


def kernel(**inputs):
    out, _ = _run(inputs, trace=False)
    return out


# revision 6
# speedup vs baseline: 1.2179x; 1.0242x over previous
"""BinaryXnorExceptOutliersLinear on 8 Trainium2 NeuronCores.

Reference math:
    mask, bscale from global kth-value quantiles of w
    w_q  = per-row asymmetric 8-bit fake quant of w
    w_sim = mask ? w_q : sign(w_q)*bscale
    out  = x @ w_sim.T + bias

Strategy: the weight transform is data-independent of x, so it is done on
the host (numpy, f32, op-for-op like the reference). The device kernel is
a DoubleRow fp8(e4m3) GEMM: per out-row o, codes = w_sim[o,:]/s_o with
s_o = bscale/nb_o and nb_o the largest e4m3-exact value such that
max|codes| <= 240. Non-outliers (+-bscale, 95% of weights) encode EXACTLY
as +-nb_o; only outliers carry e4m3 rounding (~1.3e-2 output rel err).
x is split hi+lo into two fp8 parts packed side by side in the stationary
operand (64 of 128 stationary columns -> DoubleRow max), recovering ~fp16
x precision at zero PE cost. Each core holds its full 8 MiB weight shard
resident in SBUF (64 KiB/partition), streamed via large dependency-free
DMAs on both HW DGE queues; matmuls chase the stream per 256-row k-pair
chunk. Output columns are processed in 2 groups of 512 so group 0's
PSUM drain (DVE copy + DMA out) overlaps group 1's matmuls.
"""
import sys

sys.path.insert(0, "/opt/trn_rl_repo")

import numpy as np
import ml_dtypes
from contextlib import ExitStack

import bass_rust
import concourse.bass as bass
import concourse.mybir as mybir
import concourse.tile as tile
from concourse.bass_utils import run_bass_kernel_spmd

# ---------------------------------------------------------------------------
OUT_F = 8192
IN_F = 8192
BATCH = 32
N_CORES = 8
ROWS_PER_CORE = OUT_F // N_CORES      # 1024
P = 128
CP = IN_F // (2 * P)                   # 32 k-pair chunks of 256
G = 4                                  # output column groups per core
GN = ROWS_PER_CORE // G                # 256 cols per group
OUTLIER_FRACTION = 0.05
F8MAX = 240.0                          # trn float8e4 (IEEE e4m3) max normal

f32 = mybir.dt.float32
f8 = mybir.dt.float8e4
F8NP = ml_dtypes.float8_e4m3

# ---------------------------------------------------------------------------
# walrus compatibility


def _prepare_for_walrus(nc):
    mybir.codegen_inst_isa_subclasses(nc)
    ctr = 0
    for bb in nc.main_func.blocks:
        new = []
        for inst in bb.instructions:
            si = inst.sync_info
            if si is not None and len(si.on_wait) > 1:
                waits = list(si.on_wait)
                for w in waits[:-1]:
                    nop = bass_rust.InstNoOp(
                        name=f"I-wsplit-{ctr}", engine=inst.engine
                    )
                    ctr += 1
                    nop.sync_info = mybir.SyncInfo(on_wait=[w], on_update=[])
                    try:
                        nc.register_instruction(nop, overwrite=True)
                    except Exception:
                        pass
                    new.append(nop)
                si.on_wait = [waits[-1]]
            new.append(inst)
        bb.instructions = new
    return nc


# ---------------------------------------------------------------------------
# device program
#
# psum_g[64, 512] = sum_c  xt[:, c].T @ wt[:, g*32+c]   (DoubleRow, K=256/chunk)
#   stationary xt chunk [128, 2, 64]: cols 0:32 x_hi, 32:64 x_lo
#   moving    wt chunk [128, 2, 512]
# y[64, 1024]: rows 0:32 hi part, 32:64 lo part; host adds halves, applies
# per-col scale s and bias.

# weight DMA pieces in units of 64 KiB (g,c) chunks; few big pieces so the
# two HW DGE queues saturate quickly (configs cost ~0.7us of sequencer time
# each), tapered tail so the final matmuls/drain start ASAP.  Alternates
# SP/ACT; byte-balanced with x (512 KiB) leading on ACT.
SCHEDW = [32, 32, 20, 20, 12, 8, 4]
assert sum(SCHEDW) == G * CP


def _build_nc():
    nc = bass.Bass()
    # host layouts (per-partition contiguous):
    #   wP[p, (g*32+c)*1024 + i*512 + n] = code[g*512+n, c*256+i*128+p]
    #   xS[p, c*128 + i*64 + m]          = xhl[m, c*256+i*128+p]
    wP = nc.dram_tensor("wP", [P, G * CP * 2 * GN], f8, kind="ExternalInput")
    xS = nc.dram_tensor("xS", [P, CP * 2 * 64], f8, kind="ExternalInput")
    y = nc.dram_tensor("y", [64, ROWS_PER_CORE], f32, kind="ExternalOutput")

    PM = mybir.MatmulPerfMode.DoubleRow
    A = mybir.AluOpType

    with tile.TileContext(nc) as tc, ExitStack() as ctx:
        xpool = ctx.enter_context(tc.tile_pool(name="x", bufs=1))
        wpool = ctx.enter_context(tc.tile_pool(name="w", bufs=len(SCHEDW)))
        opool = ctx.enter_context(tc.tile_pool(name="o", bufs=1))
        psum = ctx.enter_context(tc.tile_pool(name="ps", bufs=1, space="PSUM"))

        # x: one piece, first config on the ACT queue
        xt = xpool.tile([P, CP, 2, 64], f8)
        nc.scalar.dma_start(xt[:], xS[:])

        # weight pieces: dependency-free, alternate HW DGE queues
        wts = []           # (gc_start, gc_end, tile)
        c = 0
        for k, wn in enumerate(SCHEDW):
            wt = wpool.tile([P, wn, 2, GN], f8)
            eng = nc.sync if k % 2 == 0 else nc.scalar
            eng.dma_start(wt[:], wP[:, c * 2 * GN:(c + wn) * 2 * GN])
            wts.append((c, c + wn, wt))
            c += wn

        def xchunk(c):
            return xt[:, c]

        def wchunk(gc):
            for a, b, wt in wts:
                if a <= gc < b:
                    return wt[:, gc - a]
            raise AssertionError

        for g in range(G):
            ps = psum.tile([64, GN], f32, tag=f"ps{g}")
            for c in range(CP):
                nc.tensor.matmul(ps[:], xchunk(c), wchunk(g * CP + c),
                                 start=(c == 0), stop=(c == CP - 1),
                                 perf_mode=PM)
            ot = opool.tile([64, GN], f32, tag=f"ot{g}")
            nc.vector.tensor_scalar(ot[:], ps[:], 0.0, None, A.add)
            eng = nc.sync if g % 2 == 0 else nc.scalar
            eng.dma_start(y[:, g * GN:(g + 1) * GN], ot[:])

    _prepare_for_walrus(nc)
    return nc


_NC_CACHE = None


def _get_nc():
    global _NC_CACHE
    if _NC_CACHE is None:
        _NC_CACHE = _build_nc()
    return _NC_CACHE


# ---------------------------------------------------------------------------
# host precompute: reference weight transform + e4m3 encoding


def _host_wsim(weight):
    w = np.ascontiguousarray(weight, dtype=np.float32)
    n = w.size
    k_lo = int(n * OUTLIER_FRACTION / 2)
    k_hi = int(n * (1.0 - OUTLIER_FRACTION / 2))
    part = np.partition(w.reshape(-1), [k_lo - 1, k_hi - 1])
    lo = np.float32(part[k_lo - 1])
    hi = np.float32(part[k_hi - 1])
    mask = (w < lo) | (w > hi)
    keep = ~mask
    bscale = np.float32(
        np.sum(np.abs(w) * keep, dtype=np.float32)
        / np.sum(keep, dtype=np.float32)
    )
    # per-row asymmetric 8-bit fake quant, f32 op-for-op like the reference
    w_min = w.min(1, keepdims=True).astype(np.float32)
    w_max = w.max(1, keepdims=True).astype(np.float32)
    rng = (w_max - w_min).astype(np.float32)
    zp = np.round(w_min - np.float32(128.0) * rng / np.float32(255.0)).astype(
        np.float32)
    q = (w - zp).astype(np.float32)
    q = (q * np.float32(255.0)).astype(np.float32)
    q = (q / rng).astype(np.float32)
    q = np.clip(np.round(q), np.float32(0.0), np.float32(255.0)).astype(
        np.float32)
    w_q = (q * (rng / np.float32(255.0)) + zp).astype(np.float32)
    w_sim = np.where(mask, w_q, np.sign(w_q) * bscale).astype(np.float32)
    return w_sim, bscale


def _snap_down_f8(v):
    """Largest e4m3-exact value <= v (v positive normal)."""
    c = v.astype(F8NP)
    cf = c.astype(np.float32)
    bits = c.view(np.uint8)
    bits = np.where(cf > v, bits - 1, bits)
    return bits.view(F8NP).astype(np.float32)


def _encode_f8(w_sim, bscale):
    M = np.abs(w_sim).max(1)
    nb_t = (np.float32(F8MAX) * bscale / M * np.float32(0.999)).astype(
        np.float32)
    nb = _snap_down_f8(nb_t)
    s = (bscale / nb).astype(np.float32)
    codes = np.clip(w_sim / s[:, None], -F8MAX, F8MAX).astype(F8NP)
    return codes, s


def _run(inputs, trace=False):
    x, weight, bias = inputs["x"], inputs["weight"], inputs["bias"]
    w_sim, bscale = _host_wsim(weight)
    codes, s = _encode_f8(w_sim, bscale)

    x2 = np.ascontiguousarray(x, dtype=np.float32).reshape(BATCH, IN_F)
    x_hi = x2.astype(F8NP).astype(np.float32)
    x_lo = (x2 - x_hi).astype(F8NP)
    # xS[p, c*128 + i*64 + m]; m = h*32 + b; k = c*256 + i*128 + p
    st = np.stack([x_hi.astype(F8NP), x_lo], axis=0)   # [h, b, k]
    st = st.reshape(2, BATCH, CP, 2, P)                # [h, b, c, i, p]
    xSv = np.ascontiguousarray(st.transpose(4, 2, 3, 0, 1)).reshape(
        P, CP * 2 * 64)

    nc = _get_nc()
    in_maps = []
    for cid in range(N_CORES):
        sl = slice(cid * ROWS_PER_CORE, (cid + 1) * ROWS_PER_CORE)
        # wP[p, g, c, i, n] = codes_core[g*512+n, c*256+i*128+p]
        cc = codes[sl].reshape(G, GN, CP, 2, P)         # [g, n, c, i, p]
        wPc = np.ascontiguousarray(cc.transpose(4, 0, 2, 3, 1)).reshape(
            P, G * CP * 2 * GN)
        in_maps.append({"wP": wPc, "xS": xSv})
    res = run_bass_kernel_spmd(
        nc, in_maps, core_ids=list(range(N_CORES)), trace=trace
    )
    ys = np.concatenate([r["y"][0:32] + r["y"][32:64] for r in res.results],
                        axis=1)                          # [32, 8192]
    out = (ys * s[None, :] + np.asarray(bias, np.float32)[None, :]).reshape(
        BATCH, 1, OUT_F).astype(np.float32)
    return out, res


def kernel(**inputs):
    out, _ = _run(inputs, trace=False)
    return out
